# revision 31
# baseline (speedup 1.0000x reference)
"""Trainium2 Bass kernel for a 12-head attention block with post-softmax
additive per-head bias.

    qkv = x @ W_qkv                          x: [64, 196, 768]
    attn = softmax(q k^T / 8) + static_a     (bias added AFTER softmax)
    out = (attn @ v) @ W_proj + b_proj

Sharding: data-parallel over batch across 8 NeuronCores (8 batches each).
No collectives needed. Weights are replicated; x is passed transposed
([768, 1568] per core) so the contraction dim lands on SBUF partitions
without any on-chip transpose. Compute dtype bf16 (PE 1 cyc/row vs 4 for
f32), f32 PSUM accumulation.

Per-core dataflow (B=8 local batches, H=12 heads, D=64, N=196):
  qkT[h]  = [q_h^T | k_h^T] over tokens   (lhsT = W_qkv cols, rhs = x^T)
  v       = x @ W_v                        (lhsT = x^T chunks, rhs = W_v)
  AV      = A_h @ v   (per-head bias term, batched, written into O tiles)
  S^T     = k @ q^T   per (b, h)  -> exp (ACT, scale=1/8) -> P^T (bf16)
  U'      = P @ [v | 1]  (matmul with ones column gives row-sums for free)
  O       = U * (1/r) + AV   (DVE)
  O^T     via PE transpose -> attn_outT [768, 1568]
  out     = attn_out @ W_proj + b_proj  (bias via K=1 ones matmul)
"""

import os
import sys

_TRN_REPO = "/opt/trn_rl_repo"
if _TRN_REPO not in sys.path:
    sys.path.insert(0, _TRN_REPO)

import numpy as np
import ml_dtypes

import concourse.bass as bass
import concourse.tile as tile
from concourse import bacc, mybir
from concourse.bass import MemorySpace
from concourse.bass_utils import run_bass_kernel_spmd
from concourse.masks import make_identity

BF16 = mybir.dt.bfloat16
F32 = mybir.dt.float32

N_CORES = 8
BATCH = 64
B = BATCH // N_CORES  # 8 local batches per core
H = 12
D = 64
N = 196
C = 768
T = B * N  # 1568 local tokens
KC = 6  # contraction chunks of 128 over C=768
SCALE = D ** -0.5  # 0.125

# token chunks of 128 over T
MCS = [(i * 128, min(128, T - i * 128)) for i in range((T + 127) // 128)]
# per-batch row chunks over N=196
NCH = [(0, 128), (128, 68)]

AluOp = mybir.AluOpType
ActFn = mybir.ActivationFunctionType

# debug: stop after a given phase ('C', 'CP', 'D', 'E', 'all')
_STOP = os.environ.get("K_STOP", "all")
_DSUB = os.environ.get("K_DSUB", "full")  # 'st' | 'u' | 'full'
_ORDER = ["C", "CP", "D", "E", "all"]


def _runs(phase):
    return _ORDER.index(_STOP) >= _ORDER.index(phase)


def _emit(nc: bass.Bass):
    xt_d = nc.declare_dram_parameter("xt", [C, T], BF16, isOutput=False)
    wqkv_d = nc.declare_dram_parameter("wqkv", [C, 3 * C], BF16, isOutput=False)
    at_d = nc.declare_dram_parameter("at", [H, N, N], BF16, isOutput=False)
    wproj_d = nc.declare_dram_parameter("wproj", [C, C], BF16, isOutput=False)
    bproj_d = nc.declare_dram_parameter("bproj", [1, C], BF16, isOutput=False)
    out_d = nc.declare_dram_parameter("out", [T, C], F32, isOutput=True)

    with tile.TileContext(nc) as tc:
        from contextlib import ExitStack

        with ExitStack() as stk:
            const = stk.enter_context(tc.tile_pool(name="const", bufs=1))
            big = stk.enter_context(tc.tile_pool(name="big", bufs=1))
            ptp = stk.enter_context(tc.tile_pool(name="ptp", bufs=3))
            small = stk.enter_context(tc.tile_pool(name="small", bufs=2))
            outst = stk.enter_context(tc.tile_pool(name="outst", bufs=2))
            stage = stk.enter_context(tc.tile_pool(name="stage", bufs=2))

            # ---- constants ----
            ident = const.tile([128, 128], BF16)
            make_identity(nc, ident)
            zbias = const.tile([128, 1], F32)
            nc.vector.memset(zbias, 0.0)
            # b_proj broadcast to all 128 partitions (partition-step-0 DMA)
            bias_bc = const.tile([128, C], BF16)
            bproj_ap = bass.AP(
                tensor=bproj_d.ap().tensor, offset=0, ap=[[0, 128], [1, C]]
            )
            nc.gpsimd.dma_start(out=bias_bc, in_=bproj_ap)
            wproj_sb = const.tile([128, KC, C], BF16)
            for kc in range(KC):
                nc.sync.dma_start(
                    out=wproj_sb[:, kc, :], in_=wproj_d[kc * 128 : (kc + 1) * 128, :]
                )
            at_sb = const.tile([128, 2, H * N], BF16)
            for h in range(H):
                nc.sync.dma_start(
                    out=at_sb[:, 0, h * N : (h + 1) * N], in_=at_d[h, 0:128, :]
                )
                nc.sync.dma_start(
                    out=at_sb[0:68, 1, h * N : (h + 1) * N], in_=at_d[h, 128:196, :]
                )

            # ---- persistent big tensors ----
            # q^T / k^T with head pairs packed on partitions: head 2c on
            # partitions 0-63, head 2c+1 on 64-127 (PE needs lhsT and rhs at
            # the same base partition, so q and k live in separate tiles).
            qt_sb = big.tile([128, KC, T], BF16)
            kt_sb = big.tile([128, KC, T], BF16)
            v_sb = big.tile([128, B, 2, H * 65], BF16)  # v with ones col per head
            o_sb = big.tile([128, B, 2, C], BF16)  # attention out (pre-transpose)

            for b in range(B):
                for mc, (moff, mlen) in enumerate(NCH):
                    vv = v_sb[0:mlen, b, mc, :].rearrange("p (h x) -> p h x", h=H)
                    nc.vector.memset(vv[:, :, 64:65], 1.0)

            # ---- phase A loads (scoped so SBUF is reusable later) ----
            with tc.tile_pool(name="inbuf", bufs=1) as inbuf:
                xt_sb = inbuf.tile([128, KC, T], BF16)
                wqkv_sb = inbuf.tile([128, KC, 3 * C], BF16)
                for kc in range(KC):
                    nc.sync.dma_start(
                        out=xt_sb[:, kc, :], in_=xt_d[kc * 128 : (kc + 1) * 128, :]
                    )
                    nc.sync.dma_start(
                        out=wqkv_sb[:, kc, :],
                        in_=wqkv_d[kc * 128 : (kc + 1) * 128, :],
                    )

                # ---- phase B: q^T / k^T = W_{q,k}^T @ x^T, head-pair packed ----
                NSL_B = 392  # 4 slices of 392 f32 = one PSUM bank each
                with tc.tile_pool(name="qkps", bufs=6, space=MemorySpace.PSUM) as qkps:
                    for c in range(KC):
                        for dst, sec in ((qt_sb, 0), (kt_sb, C)):
                            for ns in range(4):
                                ps = qkps.tile([128, NSL_B], F32, tag="qk")
                                for kc in range(KC):
                                    nc.tensor.matmul(
                                        ps,
                                        lhsT=wqkv_sb[
                                            :, kc, sec + c * 128 : sec + (c + 1) * 128
                                        ],
                                        rhs=xt_sb[:, kc, ns * NSL_B : (ns + 1) * NSL_B],
                                        start=(kc == 0),
                                        stop=(kc == KC - 1),
                                    )
                                nc.vector.tensor_copy(
                                    dst[:, c, ns * NSL_B : (ns + 1) * NSL_B], ps
                                )

                # ---- phase C: v = x @ W_v, per-batch row chunks ----
                with tc.tile_pool(name="vps", bufs=4, space=MemorySpace.PSUM) as vps:
                    for b in range(B):
                        for vmc, (voff, vlen) in enumerate(NCH):
                            for ns in range(2):  # 2 slices of 384 cols
                                ps = vps.tile([128, 384], F32, tag="v")
                                for kc in range(KC):
                                    nc.tensor.matmul(
                                        ps[0:vlen, :],
                                        lhsT=xt_sb[
                                            :, kc, b * N + voff : b * N + voff + vlen
                                        ],
                                        rhs=wqkv_sb[
                                            :,
                                            kc,
                                            2 * C + ns * 384 : 2 * C + (ns + 1) * 384,
                                        ],
                                        start=(kc == 0),
                                        stop=(kc == KC - 1),
                                    )
                                dst = v_sb[0:vlen, b, vmc, :].rearrange(
                                    "p (h x) -> p h x", h=H
                                )
                                src = ps[0:vlen, :].rearrange("p (h c) -> p h c", h=6)
                                nc.vector.tensor_copy(
                                    dst[:, ns * 6 : (ns + 1) * 6, 0:64], src
                                )

            # attn-out-transposed lives from phase E on; opened after the
            # inbuf pool closes so it reuses that SBUF space
            aotp = stk.enter_context(tc.tile_pool(name="aotp", bufs=1))
            aot_sb = aotp.tile([128, KC, T], BF16)

            # ---- phase C': AV = A_h @ v, written into o_sb ----
            with tc.tile_pool(name="avps", bufs=2, space=MemorySpace.PSUM) as avps:
                for b in range(B if _runs("CP") else 0):
                    for nc_i, (noff, nlen) in enumerate(NCH):
                        av = avps.tile([128, 1024], F32, tag="av")
                        for h in range(H):
                            aoff = (h // 8) * 512 + (h % 8) * 64
                            for mc, (moff, mlen) in enumerate(NCH):
                                nc.tensor.matmul(
                                    av[0:nlen, aoff : aoff + 64],
                                    lhsT=at_sb[0:mlen, mc, h * N + noff : h * N + noff + nlen],
                                    rhs=v_sb[0:mlen, b, mc, h * 65 : h * 65 + 64],
                                    start=(mc == 0),
                                    stop=(mc == 1),
                                )
                        # ACT copies PSUM -> o_sb (bf16)
                        nc.scalar.copy(
                            o_sb[0:nlen, b, nc_i, 0:512], av[0:nlen, 0:512]
                        )
                        nc.scalar.copy(
                            o_sb[0:nlen, b, nc_i, 512:768], av[0:nlen, 512:768]
                        )

            # ---- phase D: S^T -> exp -> U' -> O (software-pipelined by b) ----
            with (
                tc.tile_pool(name="stps", bufs=2, space=MemorySpace.PSUM) as stps,
                tc.tile_pool(name="ups", bufs=2, space=MemorySpace.PSUM) as ups,
            ):
                pt_tiles = {}

                def emit_st_exp(b):
                    boff = b * N
                    # Stage the odd heads' q^T/k^T (stored on partitions
                    # 64-127) down to base partition 0 via SBUF->SBUF DMA.
                    # A matmul at base partition 64 followed by one at base 0
                    # with 64-row tiles crashes the hardware, so every S^T
                    # matmul must run at base partition 0.
                    stq = stage.tile([64, KC, N], BF16, tag="stq")
                    stk = stage.tile([64, KC, N], BF16, tag="stk")
                    nc.sync.dma_start(
                        out=stq, in_=qt_sb[64:128, :, boff : boff + N]
                    )
                    nc.sync.dma_start(
                        out=stk, in_=kt_sb[64:128, :, boff : boff + N]
                    )
                    for mc, (moff, mlen) in enumerate(NCH):
                        pt = ptp.tile([128, H * N], BF16, tag="pt")
                        pt_tiles[(b, mc)] = pt
                        for hg in range(3):  # 4 heads per PSUM group (2 banks)
                            ps = stps.tile([128, 1024], F32, tag="st")
                            for hh in range(4):
                                h = hg * 4 + hh
                                off = (hh // 2) * 512 + (hh % 2) * 196
                                if h % 2 == 0:
                                    lhsT = kt_sb[
                                        0:64,
                                        h // 2,
                                        boff + moff : boff + moff + mlen,
                                    ]
                                    rhs = qt_sb[0:64, h // 2, boff : boff + N]
                                else:
                                    lhsT = stk[0:64, h // 2, moff : moff + mlen]
                                    rhs = stq[0:64, h // 2, :]
                                nc.tensor.matmul(
                                    ps[0:mlen, off : off + 196],
                                    lhsT=lhsT,
                                    rhs=rhs,
                                    start=True,
                                    stop=True,
                                )
                            src = (
                                ps.rearrange("p (k x) -> p k x", k=2)[0:mlen, :, 0:392]
                                .rearrange("p k (h n) -> p k h n", h=2)
                            )
                            dst = pt[
                                0:mlen, hg * 4 * N : (hg + 1) * 4 * N
                            ].rearrange("p (k h n) -> p k h n", k=2, h=2)
                            nc.scalar.activation(
                                dst, src, ActFn.Exp, bias=zbias[0:mlen, :], scale=SCALE
                            )

                def emit_uo(b):
                    if _DSUB == "st":
                        return
                    for nc_i, (noff, nlen) in enumerate(NCH):
                        up = ups.tile([128, 1024], F32, tag="up")
                        for h in range(H):
                            uoff = (h // 6) * 512 + (h % 6) * 65
                            for mc, (moff, mlen) in enumerate(NCH):
                                pt = pt_tiles[(b, mc)]
                                nc.tensor.matmul(
                                    up[0:nlen, uoff : uoff + 65],
                                    lhsT=pt[0:mlen, h * N + noff : h * N + noff + nlen],
                                    rhs=v_sb[0:mlen, b, mc, h * 65 : h * 65 + 65],
                                    start=(mc == 0),
                                    stop=(mc == 1),
                                )
                        if _DSUB == "u":
                            continue
                        # O = U * (1/r) + AV
                        upv = up.rearrange("p (k x) -> p k x", k=2)[0:nlen, :, 0:390]
                        upv = upv.rearrange("p k (h x) -> p k h x", h=6)
                        rec = small.tile([128, H], F32, tag="rec")
                        recv = rec.rearrange("p (k h) -> p k h", k=2)[0:nlen, :, :, None]
                        nc.vector.reciprocal(recv, upv[:, :, :, 64:65])
                        tmp = small.tile([128, C], F32, tag="tmp")
                        tmpv = tmp[0:nlen, :].rearrange(
                            "p (k h c) -> p k h c", k=2, h=6
                        )
                        nc.vector.tensor_tensor(
                            tmpv,
                            upv[:, :, :, 0:64],
                            recv.to_broadcast((nlen, 2, 6, 64)),
                            AluOp.mult,
                        )
                        nc.vector.tensor_tensor(
                            o_sb[0:nlen, b, nc_i, :],
                            tmp[0:nlen, :],
                            o_sb[0:nlen, b, nc_i, :],
                            AluOp.add,
                        )

                for b in range(B if _runs("D") else 0):
                    emit_st_exp(b)
                    if b > 0:
                        emit_uo(b - 1)
                if _runs("D"):
                    emit_uo(B - 1)

            # ---- phase E: transpose O into aot_sb ----
            with tc.tile_pool(name="tps", bufs=4, space=MemorySpace.PSUM) as tps:
                for b in range(B if _runs("E") else 0):
                    for nc_i, (noff, nlen) in enumerate(NCH):
                        for hp in range(KC):
                            tp = tps.tile([128, 128], BF16, tag="tp")
                            nc.tensor.transpose(
                                tp[:, 0:nlen],
                                in_=o_sb[0:nlen, b, nc_i, hp * 128 : (hp + 1) * 128],
                                identity=ident[0:nlen, 0:nlen],
                            )
                            nc.vector.tensor_copy(
                                aot_sb[
                                    :, hp, b * N + noff : b * N + noff + nlen
                                ],
                                tp[:, 0:nlen],
                            )

            # ---- phase F: out = attn_out @ W_proj + b_proj ----
            with tc.tile_pool(name="pps", bufs=2, space=MemorySpace.PSUM) as pps:
                for mc, (moff, mlen) in enumerate(MCS if _STOP == "all" else []):
                    pp = pps.tile([128, 1024], F32, tag="pp")
                    for nsl, nw in ((0, 512), (512, 256)):
                        for kc in range(KC):
                            nc.tensor.matmul(
                                pp[0:mlen, nsl : nsl + nw],
                                lhsT=aot_sb[:, kc, moff : moff + mlen],
                                rhs=wproj_sb[:, kc, nsl : nsl + nw],
                                start=(kc == 0),
                                stop=(kc == KC - 1),
                            )
                    ot = outst.tile([128, C], F32, tag="ot")
                    nc.vector.tensor_tensor(
                        ot[0:mlen, :],
                        pp[0:mlen, 0:768],
                        bias_bc[0:mlen, :],
                        AluOp.add,
                    )
                    nc.sync.dma_start(
                        out=out_d[moff : moff + mlen, :], in_=ot[0:mlen, :]
                    )

            if _STOP != "all":
                # debug probes: dump intermediates into `out` rows
                probes = [
                    ("C", v_sb[0:128, 0, 0, 0:768], 0),
                    ("C", qt_sb[0:128, 0, 0:768], 128),
                    ("C", kt_sb[0:128, 0, 0:768], 256),
                    ("CP", o_sb[0:128, 0, 0, :], 384),
                    ("D", o_sb[0:128, 1, 0, :], 512),
                    ("E", aot_sb[0:128, 0, 0:768], 640),
                ]
                for ph, src, row in probes:
                    if not _runs(ph):
                        continue
                    pb = outst.tile([128, C], F32, tag="ot")
                    nc.vector.tensor_copy(pb, src)
                    nc.sync.dma_start(out=out_d[row : row + 128, :], in_=pb)

    return nc


def _fix_wait_counts(nc, limits=None):
    """Walrus's tiled-matmul (S3D3) struct accepts only one sync-wait per
    instruction, but Tile's scheduler sometimes attaches two. Move the extra
    waits onto earlier wait-free instructions of the same engine stream —
    waiting earlier for the same semaphore threshold is always legal, as long
    as we never move a wait past an instruction that increments the same
    semaphore (the threshold could then depend on a blocked increment).
    """
    if limits is None:
        limits = {
            mybir.InstMatmult: 1,
            mybir.InstLdweights: 1,
            mybir.InstTensorCopy: 1,
            mybir.InstTensorTensor: 1,
            mybir.InstActivation: 1,
            mybir.InstReciprocal: 1,
            mybir.InstMemset: 1,
            mybir.InstDMACopy: 1,
        }
    fix_id = [0]
    for fn in nc.m.functions:
        for bb in fn.blocks:
            insts = bb.instructions
            inserts = []  # (pos, nop) applied at the end
            for pos, ins in enumerate(insts):
                lim = None
                for klass, kl in limits.items():
                    if isinstance(ins, klass):
                        lim = kl
                        break
                if lim is None:
                    continue
                si = getattr(ins, "sync_info", None)
                if si is None or not si.on_wait or len(si.on_wait) <= lim:
                    continue
                keep = list(si.on_wait)
                moved = []
                for w in list(keep):
                    if len(keep) <= lim:
                        break
                    # scan backward through same-engine instructions
                    p = pos - 1
                    placed = False
                    while p >= 0:
                        c = insts[p]
                        if getattr(c, "engine", None) != ins.engine:
                            p -= 1
                            continue
                        csi = getattr(c, "sync_info", None)
                        # does c update w's semaphore? then stop (unsafe past it)
                        if csi is not None and any(
                            u.id == w.id for u in (csi.on_update or [])
                        ):
                            break
                        if (csi is None or not csi.on_wait) and not isinstance(
                            c, mybir.InstEventSemaphore
                        ):
                            if csi is None:
                                c.sync_info = mybir.SyncInfo(
                                    on_wait=[w], on_update=[]
                                )
                            else:
                                csi.on_wait = [w]
                            keep.remove(w)
                            moved.append((ins.name, c.name, w.ant_name))
                            placed = True
                            break
                        p -= 1
                    if not placed:
                        # no same-engine host available: insert a NoOp
                        # carrying this wait right before the instruction
                        fix_id[0] += 1
                        nop = mybir.InstNoOp(
                            name=f"I-waitfix-{fix_id[0]}",
                            engine=ins.engine,
                            ins=[],
                            outs=[],
                            sync_info=mybir.SyncInfo(on_wait=[w], on_update=[]),
                        )
                        inserts.append((pos, nop))
                        keep.remove(w)
                si.on_wait = keep
                if len(keep) > lim:
                    raise RuntimeError(
                        f"could not reduce waits on {ins.name}: {keep}"
                    )
            for pos, nop in reversed(inserts):
                insts.insert(pos, nop)
    return nc


_CACHE: dict = {}


def _get_module():
    if "nc" not in _CACHE:
        nc = bacc.Bacc(None, target_bir_lowering=False)
        _emit(nc)
        nc.compile()
        _CACHE["nc"] = nc
    return _CACHE["nc"]


_last_results = None


def kernel(x, W_qkv, static_a, W_proj, b_proj):
    global _last_results
    bf = ml_dtypes.bfloat16
    x = np.asarray(x, dtype=np.float32)
    wqkv_b = np.asarray(W_qkv, dtype=np.float32).astype(bf)
    at_b = np.ascontiguousarray(
        np.transpose(np.asarray(static_a, dtype=np.float32)[0], (0, 2, 1))
    ).astype(bf)
    wproj_b = np.asarray(W_proj, dtype=np.float32).astype(bf)
    bproj_b = np.asarray(b_proj, dtype=np.float32).reshape(1, C).astype(bf)

    in_maps = []
    for i in range(N_CORES):
        shard = x[i * B : (i + 1) * B].reshape(T, C)
        xt_b = np.ascontiguousarray(shard.T).astype(bf)
        in_maps.append(
            dict(xt=xt_b, wqkv=wqkv_b, at=at_b, wproj=wproj_b, bproj=bproj_b)
        )

    nc = _get_module()
    res = run_bass_kernel_spmd(nc, in_maps, core_ids=list(range(N_CORES)))
    _last_results = res
    out = np.concatenate(
        [np.asarray(r["out"]).reshape(B, N, C) for r in res.results], axis=0
    )
    return out.astype(np.float32)


# revision 55
# speedup vs baseline: 1.2987x; 1.2987x over previous
"""Trainium2 Bass kernel for a 12-head attention block with post-softmax
additive per-head bias.

    qkv = x @ W_qkv                          x: [64, 196, 768]
    attn = softmax(q k^T / 8) + static_a     (bias added AFTER softmax)
    out = (attn @ v) @ W_proj + b_proj

Sharding: data-parallel over batch across 8 NeuronCores (8 batches each).
No collectives needed. Weights are replicated; x is passed transposed
([768, 1568] per core) so the contraction dim lands on SBUF partitions
without any on-chip transpose. Compute dtype bf16 (PE 1 cyc/row vs 4 for
f32), f32 PSUM accumulation.

Per-core dataflow, software-pipelined over the 8 local batches b:
  qkT(b)  = W_{q,k}^T @ x_b^T      (head-pair packed on partitions)
  v(b)    = x_b @ W_v              (65-stride layout with a ones column)
  S^T(b)  = k @ q^T  -> exp (ACT, scale=1/8, fused PSUM->SBUF) -> P^T
  AV(b)   = A_h @ v                (per-head bias term, ACT-copied to O)
  U'(b-1) = P @ [v|1]              (ones column gives softmax row sums)
  O(b-1)  = U * (1/r) + AV         (DVE)
  O^T(b-1) via PE transpose -> attn_outT
  out     = attn_out @ W_proj + b_proj  (bias via broadcast-DMA + DVE add)

The b-1 stages overlap ACT's exp(b), keeping the TensorEngine busy.
"""

import os
import sys

_TRN_REPO = "/opt/trn_rl_repo"
if _TRN_REPO not in sys.path:
    sys.path.insert(0, _TRN_REPO)

import numpy as np
import ml_dtypes

import concourse.bass as bass
import concourse.tile as tile
from concourse import bacc, mybir
from concourse.bass import MemorySpace
from concourse.bass_utils import run_bass_kernel_spmd
from concourse.masks import make_identity

BF16 = mybir.dt.bfloat16
F32 = mybir.dt.float32

N_CORES = 8
BATCH = 64
B = BATCH // N_CORES  # 8 local batches per core
H = 12
D = 64
N = 196
C = 768
T = B * N  # 1568 local tokens
KC = 6  # contraction chunks of 128 over C=768
SCALE = D ** -0.5  # 0.125

# token chunks of 128 over T (for the projection)
MCS = [(i * 128, min(128, T - i * 128)) for i in range((T + 127) // 128)]
# per-batch row chunks over N=196
NCH = [(0, 128), (128, 68)]

AluOp = mybir.AluOpType
ActFn = mybir.ActivationFunctionType


def _emit(nc: bass.Bass):
    # xt: per-batch partition-major x^T blocks: xt[b, p, kc*N+n] = x[b, n, kc*128+p]
    # at: partition-major A^T: at[mc, p, h*N+n] = A[h, n, mc*128+p]
    xt_d = nc.declare_dram_parameter("xt", [B, 128, KC * N], BF16, isOutput=False)
    wqkv_d = nc.declare_dram_parameter("wqkv", [C, 3 * C], BF16, isOutput=False)
    at_d = nc.declare_dram_parameter("at", [2, 128, H * N], BF16, isOutput=False)
    wproj_d = nc.declare_dram_parameter("wproj", [C, C], BF16, isOutput=False)
    bproj_d = nc.declare_dram_parameter("bproj", [1, C], BF16, isOutput=False)
    out_d = nc.declare_dram_parameter("out", [T, C], F32, isOutput=True)

    with tile.TileContext(nc) as tc:
        from contextlib import ExitStack

        with ExitStack() as stk:
            const = stk.enter_context(tc.tile_pool(name="const", bufs=1))
            wq = stk.enter_context(tc.tile_pool(name="wq", bufs=1))
            xtp = stk.enter_context(tc.tile_pool(name="xtp", bufs=3))
            qkp = stk.enter_context(tc.tile_pool(name="qkp", bufs=3))
            vbp = stk.enter_context(tc.tile_pool(name="vbp", bufs=3))
            obp = stk.enter_context(tc.tile_pool(name="obp", bufs=3))
            ptp = stk.enter_context(tc.tile_pool(name="ptp", bufs=4))
            stage = stk.enter_context(tc.tile_pool(name="stage", bufs=2))
            small = stk.enter_context(tc.tile_pool(name="small", bufs=3))
            outst = stk.enter_context(tc.tile_pool(name="outst", bufs=3))
            aotp = stk.enter_context(tc.tile_pool(name="aotp", bufs=1))

            # ---- constants (cheap; loaded early) ----
            ident = const.tile([128, 128], BF16)
            make_identity(nc, ident)
            zbias = const.tile([128, 1], F32)
            nc.vector.memset(zbias, 0.0)

            wqkv_sb = wq.tile([128, KC, 3 * C], BF16)
            at_sb = const.tile([128, 2, H * N], BF16)
            wproj_sb = const.tile([128, KC, C], BF16)
            bias_bc = const.tile([128, C], BF16)
            aot_sb = aotp.tile([128, KC, T], BF16)

            # per-batch rotating tiles, tracked across loop iterations
            xtb_t = {}
            qtb_t = {}
            ktb_t = {}
            vb_t = {}
            ob_t = {}
            pt_t = {}

            def emit_load_x(b):
                xtb = xtp.tile([128, KC, N], BF16, tag="xtb")
                xtb_t[b] = xtb
                nc.sync.dma_start(
                    out=xtb.rearrange("p k n -> p (k n)"), in_=xt_d[b]
                )

            def emit_qkT(b):
                xtb = xtb_t[b]
                qtb = qkp.tile([128, KC, N], BF16, tag="qtb")
                ktb = qkp.tile([128, KC, N], BF16, tag="ktb")
                qtb_t[b], ktb_t[b] = qtb, ktb
                for dst, sec in ((qtb, 0), (ktb, C)):
                    for c in range(KC):
                        ps = psA.tile([128, 512], F32, tag="pA")
                        for kc in range(KC):
                            nc.tensor.matmul(
                                ps[:, 0:N],
                                lhsT=wqkv_sb[
                                    :, kc, sec + c * 128 : sec + (c + 1) * 128
                                ],
                                rhs=xtb[:, kc, :],
                                start=(kc == 0),
                                stop=(kc == KC - 1),
                            )
                        nc.vector.tensor_copy(dst[:, c, :], ps[:, 0:N])
                # stage odd heads' q^T/k^T down to base partition 0: a 64-row
                # matmul at base partition 64 followed by one at base 0
                # crashes the hardware, so S^T always reads base-0 operands.
                stq = stage.tile([64, KC, N], BF16, tag="stq")
                stk_ = stage.tile([64, KC, N], BF16, tag="stk")
                nc.sync.dma_start(out=stq, in_=qtb[64:128, :, :])
                nc.sync.dma_start(out=stk_, in_=ktb[64:128, :, :])
                return stq, stk_

            def emit_v(b):
                xtb = xtb_t[b]
                vb = vbp.tile([128, 2, H * 65], BF16, tag="vb")
                vb_t[b] = vb
                for mc, (moff, mlen) in enumerate(NCH):
                    vv = vb[0:mlen, mc, :].rearrange("p (h x) -> p h x", h=H)
                    nc.vector.memset(vv[:, :, 64:65], 1.0)
                    for ns in range(2):
                        ps = psA.tile([128, 512], F32, tag="pA")
                        for kc in range(KC):
                            nc.tensor.matmul(
                                ps[0:mlen, 0:384],
                                lhsT=xtb[:, kc, moff : moff + mlen],
                                rhs=wqkv_sb[
                                    :, kc, 2 * C + ns * 384 : 2 * C + (ns + 1) * 384
                                ],
                                start=(kc == 0),
                                stop=(kc == KC - 1),
                            )
                        nc.vector.tensor_copy(
                            vv[:, ns * 6 : (ns + 1) * 6, 0:64],
                            ps[0:mlen, 0:384].rearrange("p (h c) -> p h c", h=6),
                        )

            def emit_st_exp(b, stq, stk_):
                qtb, ktb = qtb_t[b], ktb_t[b]
                for mc, (moff, mlen) in enumerate(NCH):
                    pt = ptp.tile([128, H * N], BF16, tag="pt")
                    pt_t[(b, mc)] = pt
                    for hg in range(3):  # 4 heads per 2-bank PSUM group
                        ps = psB.tile([128, 1024], F32, tag="pB")
                        for hh in range(4):
                            h = hg * 4 + hh
                            off = (hh // 2) * 512 + (hh % 2) * 196
                            if h % 2 == 0:
                                lhsT = ktb[0:64, h // 2, moff : moff + mlen]
                                rhs = qtb[0:64, h // 2, :]
                            else:
                                lhsT = stk_[0:64, h // 2, moff : moff + mlen]
                                rhs = stq[0:64, h // 2, :]
                            nc.tensor.matmul(
                                ps[0:mlen, off : off + 196],
                                lhsT=lhsT,
                                rhs=rhs,
                                start=True,
                                stop=True,
                            )
                        src = ps.rearrange("p (k x) -> p k x", k=2)[
                            0:mlen, :, 0:392
                        ].rearrange("p k (h n) -> p k h n", h=2)
                        dst = pt[0:mlen, hg * 4 * N : (hg + 1) * 4 * N].rearrange(
                            "p (k h n) -> p k h n", k=2, h=2
                        )
                        nc.scalar.activation(
                            dst, src, ActFn.Exp, bias=zbias[0:mlen, :], scale=SCALE
                        )

            def emit_av(b):
                vb = vb_t[b]
                ob = obp.tile([128, 2, C], BF16, tag="ob")
                ob_t[b] = ob
                for nc_i, (noff, nlen) in enumerate(NCH):
                    av = psB.tile([128, 1024], F32, tag="pB")
                    for h in range(H):
                        aoff = (h // 8) * 512 + (h % 8) * 64
                        for mc, (moff, mlen) in enumerate(NCH):
                            nc.tensor.matmul(
                                av[0:nlen, aoff : aoff + 64],
                                lhsT=at_sb[
                                    0:mlen, mc, h * N + noff : h * N + noff + nlen
                                ],
                                rhs=vb[0:mlen, mc, h * 65 : h * 65 + 64],
                                start=(mc == 0),
                                stop=(mc == 1),
                            )
                    nc.scalar.copy(ob[0:nlen, nc_i, 0:512], av[0:nlen, 0:512])
                    nc.scalar.copy(ob[0:nlen, nc_i, 512:768], av[0:nlen, 512:768])

            def emit_uo(b):
                vb = vb_t[b]
                ob = ob_t[b]
                tmps = []
                for nc_i, (noff, nlen) in enumerate(NCH):
                    # two 1-bank halves (6 heads each) so each PSUM slot
                    # frees right after its own half-size mult on DVE
                    rec = small.tile([128, H], F32, tag="rec")
                    tmp = small.tile([128, C], F32, tag="tmp")
                    for half in range(2):
                        uph = psA.tile([128, 512], F32, tag="pA")
                        for h in range(half * 6, half * 6 + 6):
                            uoff = (h % 6) * 65
                            for mc, (moff, mlen) in enumerate(NCH):
                                pt = pt_t[(b, mc)]
                                nc.tensor.matmul(
                                    uph[0:nlen, uoff : uoff + 65],
                                    lhsT=pt[
                                        0:mlen, h * N + noff : h * N + noff + nlen
                                    ],
                                    rhs=vb[0:mlen, mc, h * 65 : h * 65 + 65],
                                    start=(mc == 0),
                                    stop=(mc == 1),
                                )
                        upv = uph[0:nlen, 0:390].rearrange("p (h x) -> p h x", h=6)
                        recv = rec[0:nlen, half * 6 : half * 6 + 6, None]
                        nc.vector.reciprocal(recv, upv[:, :, 64:65])
                        nc.vector.tensor_tensor(
                            tmp[0:nlen, half * 384 : (half + 1) * 384].rearrange(
                                "p (h c) -> p h c", h=6
                            ),
                            upv[:, :, 0:64],
                            recv.to_broadcast((nlen, 6, 64)),
                            AluOp.mult,
                        )
                    tmps.append((nc_i, nlen, tmp))
                for nc_i, nlen, tmp in tmps:
                    nc.vector.tensor_tensor(
                        ob[0:nlen, nc_i, :],
                        tmp[0:nlen, :],
                        ob[0:nlen, nc_i, :],
                        AluOp.add,
                    )

            def emit_tr(b):
                ob = ob_t[b]
                for nc_i, (noff, nlen) in enumerate(NCH):
                    for hp in range(KC):
                        tp = psA.tile([128, 512], BF16, tag="pA")
                        nc.tensor.transpose(
                            tp[:, 0:nlen],
                            in_=ob[0:nlen, nc_i, hp * 128 : (hp + 1) * 128],
                            identity=ident[0:nlen, 0:nlen],
                        )
                        nc.vector.tensor_copy(
                            aot_sb[:, hp, b * N + noff : b * N + noff + nlen],
                            tp[:, 0:nlen],
                        )

            def emit_proj_chunk(mc, pps, tag="pp"):
                moff, mlen = MCS[mc]
                pp = pps.tile([128, 1024], F32, tag=tag)
                for nsl, nw in ((0, 512), (512, 256)):
                    for kc in range(KC):
                        nc.tensor.matmul(
                            pp[0:mlen, nsl : nsl + nw],
                            lhsT=aot_sb[:, kc, moff : moff + mlen],
                            rhs=wproj_sb[:, kc, nsl : nsl + nw],
                            start=(kc == 0),
                            stop=(kc == KC - 1),
                        )
                ot = outst.tile([128, C], F32, tag="ot")
                nc.vector.tensor_tensor(
                    ot[0:mlen, :],
                    pp[0:mlen, 0:768],
                    bias_bc[0:mlen, :],
                    AluOp.add,
                )
                nc.sync.dma_start(
                    out=out_d[moff : moff + mlen, :], in_=ot[0:mlen, :]
                )

            with (
                tc.tile_pool(name="psA", bufs=2, space=MemorySpace.PSUM) as psA,
                tc.tile_pool(name="psB", bufs=3, space=MemorySpace.PSUM) as psB,
            ):
                # input DMAs for batch 0, then weights in use-order
                emit_load_x(0)
                for sec in (0, C):
                    for kc in range(KC):
                        nc.sync.dma_start(
                            out=wqkv_sb[:, kc, sec : sec + C],
                            in_=wqkv_d[kc * 128 : (kc + 1) * 128, sec : sec + C],
                        )
                for kc in range(KC):
                    nc.sync.dma_start(
                        out=wqkv_sb[:, kc, 2 * C : 3 * C],
                        in_=wqkv_d[kc * 128 : (kc + 1) * 128, 2 * C : 3 * C],
                    )
                for mc in range(2):
                    nc.sync.dma_start(out=at_sb[:, mc, :], in_=at_d[mc])
                stqk = {}
                for b in range(B):
                    if b + 1 < B:
                        emit_load_x(b + 1)
                    stqk[b] = emit_qkT(b)
                    emit_v(b)
                    if b > 0:
                        emit_uo(b - 1)
                        emit_tr(b - 1)
                    emit_st_exp(b, *stqk[b])
                    emit_av(b)
                    if b == 1:
                        # projection weights stream in behind the early batches
                        for kc in range(KC):
                            nc.sync.dma_start(
                                out=wproj_sb[:, kc, :],
                                in_=wproj_d[kc * 128 : (kc + 1) * 128, :],
                            )
                        bproj_ap = bass.AP(
                            tensor=bproj_d.ap().tensor,
                            offset=0,
                            ap=[[0, 128], [1, C]],
                        )
                        nc.gpsimd.dma_start(out=bias_bc, in_=bproj_ap)
                emit_uo(B - 1)
                emit_tr(B - 1)
                # projection inside the same PSUM scope (no pool-transition
                # stall); early chunks only depend on early batches
                for mc in range(len(MCS)):
                    emit_proj_chunk(mc, psB, tag="pB")

    return nc


_CACHE: dict = {}


def _get_module():
    if "nc" not in _CACHE:
        nc = bacc.Bacc(None, target_bir_lowering=False)
        _emit(nc)
        nc.compile()
        _CACHE["nc"] = nc
    return _CACHE["nc"]


_last_results = None


def kernel(x, W_qkv, static_a, W_proj, b_proj):
    global _last_results
    bf = ml_dtypes.bfloat16
    x = np.asarray(x, dtype=np.float32)
    wqkv_b = np.asarray(W_qkv, dtype=np.float32).astype(bf)
    A = np.asarray(static_a, dtype=np.float32)[0]  # [H, N, N]
    Am = np.ascontiguousarray(A.transpose(2, 0, 1))  # [m, H, n]
    at_arr = np.zeros((2, 128, H, N), dtype=np.float32)
    at_arr[0] = Am[0:128]
    at_arr[1, 0:68] = Am[128:196]
    at_b = at_arr.reshape(2, 128, H * N).astype(bf)
    wproj_b = np.asarray(W_proj, dtype=np.float32).astype(bf)
    bproj_b = np.asarray(b_proj, dtype=np.float32).reshape(1, C).astype(bf)

    in_maps = []
    for i in range(N_CORES):
        shard = x[i * B : (i + 1) * B]  # [B, N, C]
        # [B, 128, KC*N]: xt[b, p, kc*N + n] = x[b, n, kc*128 + p]
        xt_b = np.ascontiguousarray(
            shard.transpose(0, 2, 1)
            .reshape(B, KC, 128, N)
            .transpose(0, 2, 1, 3)
            .reshape(B, 128, KC * N)
        ).astype(bf)
        in_maps.append(
            dict(xt=xt_b, wqkv=wqkv_b, at=at_b, wproj=wproj_b, bproj=bproj_b)
        )

    nc = _get_module()
    res = run_bass_kernel_spmd(nc, in_maps, core_ids=list(range(N_CORES)))
    _last_results = res
    out = np.concatenate(
        [np.asarray(r["out"]).reshape(B, N, C) for r in res.results], axis=0
    )
    return out.astype(np.float32)
